# revision 14
# baseline (speedup 1.0000x reference)
"""CapsuleCONV Trainium2 kernel v2 — few large instructions.

Layout (per core, 4 batches):
  partitions p = (d, m8, n4) = 32d + 4*m8 + n4;  m = 8q + m8, n = 8o+4h+n4
  V slab per kl: [128, (o,h,q,a,hw)] bf16 via votes matmuls
    votes: lhsT = wv[32o:32o+32 = (nn,x), (d,m8,n4)] (zero rows for nn != 4h+n4),
    rhs = X[(o,nn,x), (h,w,a2)] -> out [128, (hw, a2)] f32, 2 a-halves / bank pair
  P = V * u2 (DVE, 1 instr/kl, u2 = 0.25*ncv rearranged, n-broadcast)
  logits: L[(m8,n4)@slot o, (q2,hw)] = 4-accum matmuls over a with lhsT SL
  softmax: exp (Act) -> E slab; den via SDen matmul (m8-sum, replicated);
  recip; A = E*rd; A-rep over d via broadcast DMA; P2 = V*Arep (per o,h)
  out: out_acc[(q,d,m8), (a2,hw)] += SO32^T @ P2 chunks (PSUM accum over kl,o,h)
  LN over (a,d) via SMean matmuls + DVE; final permute via SF_d matmuls -> fin
"""
import numpy as np
import ml_dtypes
from contextlib import ExitStack

import concourse.bass as bass
import concourse.tile as tile
from concourse import bacc, mybir
from concourse._compat import with_exitstack

F32 = mybir.dt.float32
BF16 = mybir.dt.bfloat16
BF = ml_dtypes.bfloat16

B, N, H, W, DIN = 32, 32, 32, 32, 16
M, DOUT = 32, 16
KK, STRIDE = 3, 2
HO = WO = 15
HWO = HO * WO  # 225
NCORES = 8
NB = B // NCORES
SCALE = 0.25
LN_EPS = 1e-5


# ---------------------------------------------------------------- host prep
def host_prep(input_, ncv, w, gamma, beta):
    # X: [b, p=4n+x, (h, w, a)]
    inputT = np.ascontiguousarray(
        input_.reshape(B, N, H, W, 4, 4).transpose(0, 1, 5, 2, 3, 4)
        .reshape(B, 128, H * W * 4)).astype(BF)

    # u2[b, (d,m8,n4), (q,a,hw)] = 0.25 * ncv[b, 8q+m8, hw, a, d]
    nc6 = ncv.reshape(B, 4, 8, HWO, 4, 4)          # [b, q, m8, hw, a, d]
    u2 = 0.25 * nc6.transpose(0, 5, 2, 1, 4, 3)    # [b, d, m8, q, a, hw]
    u2 = np.broadcast_to(u2[:, :, :, None], (B, 4, 8, 4, 4, 4, HWO))
    u2 = np.ascontiguousarray(u2.reshape(B, 128, 4 * 4 * HWO)).astype(BF)

    # wv[(o,nn,x), (kl, h, q, (d,m8,n4))]
    w9 = w.reshape(9, N, 4, 4, M)                   # [kl, n, x, d, m]
    wv = np.zeros((4, 8, 4, 9, 2, 4, 4, 8, 4), np.float32)
    for o in range(4):
        for h in range(2):
            for n4 in range(4):
                nn = 4 * h + n4
                src = w9[:, 8 * o + nn]             # [kl, x, d, m]
                src = src.reshape(9, 4, 4, 4, 8)    # [kl, x, d, q, m8]
                wv[o, nn, :, :, h, :, :, :, n4] = src.transpose(1, 0, 3, 2, 4)
    wv = np.ascontiguousarray(wv.reshape(128, 9 * 2 * 4 * 128)).astype(BF)

    dd, m8g, n4g = np.meshgrid(np.arange(4), np.arange(8), np.arange(4),
                               indexing="ij")
    pd = (32 * dd + 4 * m8g + n4g).ravel()          # p=(d,m8,n4)

    # SL[(d,m8,n4), 4m8'+n4'] = delta
    sl = np.zeros((128, 32), np.float32)
    sl[pd, (4 * m8g + n4g).ravel()] = 1.0
    # SDen[(o,m8,n4), (o',m8',n4')] = d_o d_n4 (any m8')
    sden = np.zeros((128, 128), np.float32)
    for o in range(4):
        for m8 in range(8):
            for n4 in range(4):
                for m8p in range(8):
                    sden[32 * o + 4 * m8 + n4, 32 * o + 4 * m8p + n4] = 1.0
    # SO32[(d,m8,n4), 8d'+m8'] = d_d d_m8
    so32 = np.zeros((128, 32), np.float32)
    so32[pd, (8 * dd + m8g).ravel()] = 1.0
    # SMean[(q,d,m8), (q',d',m8')] = d_q d_m8 / 16
    smean = np.zeros((128, 128), np.float32)
    for q in range(4):
        for d in range(4):
            for m8 in range(8):
                for dp in range(4):
                    smean[32 * q + 8 * d + m8, 32 * q + 8 * dp + m8] = 1.0 / 16
    # SF[d][(q,d',m8), 8q+m8] = d_{d',d}
    sf = np.zeros((4, 128, 32), np.float32)
    for d in range(4):
        for q in range(4):
            for m8 in range(8):
                sf[d, 32 * q + 8 * d + m8, 8 * q + m8] = 1.0
    # gamma/beta cols at p=(q,d,m8): [p, a] = gamma[4a+d]
    gcol = np.zeros((128, 4), np.float32)
    bcol = np.zeros((128, 4), np.float32)
    for q in range(4):
        for d in range(4):
            for m8 in range(8):
                for a in range(4):
                    gcol[32 * q + 8 * d + m8, a] = gamma[4 * a + d]
                    bcol[32 * q + 8 * d + m8, a] = beta[4 * a + d]

    return dict(inputT=inputT, u2=u2, wv=wv,
                sl=sl.astype(BF), sden=sden.astype(BF), so32=so32.astype(BF),
                smean=smean.astype(BF), sf=sf,
                gcol=gcol, bcol=bcol)


# ------------------------------------------------------------- tile program
@with_exitstack
def build_program(ctx: ExitStack, tc: tile.TileContext, dram: dict,
                  nb=NB, kls=None, reps=1, debug=False):
    nc = tc.nc
    if kls is None:
        kls = [(k, l) for k in range(3) for l in range(3)]
    nkl = len(kls)
    nb_orig = nb
    nb = nb * reps

    const = ctx.enter_context(tc.tile_pool(name="const", bufs=1))
    xpool = ctx.enter_context(tc.tile_pool(name="xpool", bufs=1))
    vpool = ctx.enter_context(tc.tile_pool(name="vpool", bufs=1))
    ppool = ctx.enter_context(tc.tile_pool(name="ppool", bufs=1))
    p2pool = ctx.enter_context(tc.tile_pool(name="p2pool", bufs=2))
    spool = ctx.enter_context(tc.tile_pool(name="spool", bufs=1))
    mpool = ctx.enter_context(tc.tile_pool(name="mpool", bufs=2))
    vps = ctx.enter_context(tc.tile_pool(name="vps", bufs=2, space="PSUM"))
    lps = ctx.enter_context(tc.tile_pool(name="lps", bufs=2, space="PSUM"))
    ops = ctx.enter_context(tc.tile_pool(name="ops", bufs=1, space="PSUM"))

    # ---- constants
    wv_sb = const.tile([128, 9 * 2 * 4 * 128], BF16, tag="wv")
    nc.sync.dma_start(wv_sb[:], dram["wv"])
    wv_v = wv_sb[:].rearrange("p (kl h q c) -> p kl h q c", kl=9, h=2, q=4)
    sl_sb = const.tile([128, 32], BF16, tag="sl")
    nc.sync.dma_start(sl_sb[:], dram["sl"])
    sden_sb = const.tile([128, 128], BF16, tag="sden")
    nc.sync.dma_start(sden_sb[:], dram["sden"])
    so_sb = const.tile([128, 32], BF16, tag="so")
    nc.sync.dma_start(so_sb[:], dram["so32"])
    sm_sb = const.tile([128, 128], BF16, tag="sm")
    nc.sync.dma_start(sm_sb[:], dram["smean"])
    sf_sb = const.tile([128, 4 * 32], F32, tag="sf")
    nc.sync.dma_start(sf_sb[:].rearrange("p (d c) -> p d c", d=4),
                      dram["sf"].rearrange("d p c -> p d c"))
    gc_sb = const.tile([128, 4], F32, tag="gc")
    nc.sync.dma_start(gc_sb[:], dram["gcol"])
    bc_sb = const.tile([128, 4], F32, tag="bc")
    nc.sync.dma_start(bc_sb[:], dram["bcol"])
    eps_sb = const.tile([128, 1], F32, tag="eps")
    nc.vector.memset(eps_sb[:], LN_EPS)

    for bi in range(nb):
        bdi = bi % nb_orig
        xt = xpool.tile([128, 4096], BF16, tag="xt")
        nc.sync.dma_start(xt[:], dram["inputT"][bdi])
        xt_v = xt[:].rearrange("p (h w a) -> p h w a", h=32, w=32)
        u2_sb = xpool.tile([128, 4 * 4 * HWO], BF16, tag="u2")
        nc.sync.dma_start(u2_sb[:], dram["u2"][bdi])

        # out accumulator [128=(q,d,m8), a-half regions at 0 and 512]
        oacc = ops.tile([128, 1024], F32, tag="oacc", name="oacc")
        first_acc = True

        vsl = vpool.tile([128, 8 * 4 * 4 * HWO], BF16, tag="V", name="V")
        vsl_v = vsl[:].rearrange("p (o h q a hw) -> p o h q a hw",
                                 o=4, h=2, q=4, a=4)
        psl = ppool.tile([128, 8 * 4 * 4 * HWO], BF16, tag="P", name="P")
        psl_v = psl[:].rearrange("p (o h q a hw) -> p o h q a hw",
                                 o=4, h=2, q=4, a=4)
        esl = spool.tile([128, 2 * 4 * HWO], BF16, tag="E", name="E")
        esl_v = esl[:].rearrange("p (h q hw) -> p h q hw", h=2, q=4)
        asl = spool.tile([128, 2 * 4 * HWO], BF16, tag="A", name="A")
        asl_v = asl[:].rearrange("p (h q hw) -> p h q hw", h=2, q=4)
        arep = spool.tile([128, 4 * 2 * 4 * HWO], BF16, tag="Arep",
                          name="Arep")
        arep_p = arep[:].rearrange("(dd r) (o c) -> dd r o c", dd=4, o=4)
        arep_v = arep[:].rearrange("p (o h q hw) -> p o h q hw",
                                   o=4, h=2, q=4)

        for ki, (k, l) in enumerate(kls):
            kl = 3 * k + l
            # ---- votes
            for o in range(4):
                for h in range(2):
                    for q in range(4):
                        vt = vps.tile([128, 1024], F32, tag="vt", name="vt")
                        for ah in range(2):
                            nc.tensor.matmul(
                                vt[:, 512 * ah:512 * ah + 450],
                                wv_v[32 * o:32 * o + 32, kl, h, q, :],
                                xt_v[32 * o:32 * o + 32, k:k + 29:2,
                                     l:l + 29:2, 2 * ah:2 * ah + 2],
                                start=True, stop=True,
                                tile_position=(32 * o, 0))
                        # copy both halves: src (ah, hw, a2) -> dst (ah, hw, a2)
                        nc.scalar.copy(
                            vsl_v[:, o, h, q, :, :].rearrange(
                                "p (ah a2) hw -> p ah hw a2", ah=2),
                            vt[:].rearrange("p (ah c) -> p ah c", ah=2)
                            [:, :, :450].rearrange(
                                "p ah (hw a2) -> p ah hw a2", a2=2))

            if debug and bi == 0 and ki == 0:
                nc.sync.dma_start(dram["dV"], vsl[:])
            # ---- P = V * u2  (per (o,h) chunk for pipelining)
            for o in range(4):
                for h in range(2):
                    nc.vector.tensor_mul(
                        psl_v[:, o, h].rearrange("p q a hw -> p (q a hw)"),
                        vsl_v[:, o, h].rearrange("p q a hw -> p (q a hw)"),
                        u2_sb[:])

            # ---- logits: 4-accum over a; pack 4 o-slots per L tile
            for h in range(2):
                for qp in range(2):
                    lt = lps.tile([128, 512], F32, tag="lt", name="lt")
                    for o in range(4):
                        for a in range(4):
                            nc.tensor.matmul(
                                lt[32 * o:32 * o + 32, :450],
                                sl_sb[:],
                                psl_v[:, o, h, 2 * qp:2 * qp + 2, a, :],
                                start=(a == 0), stop=(a == 3),
                                tile_position=(0, 32 * o))
                    nc.scalar.activation(
                        esl_v[:, h, 2 * qp:2 * qp + 2, :],
                        lt[:, :450].rearrange("p (q2 hw) -> p q2 hw", q2=2),
                        mybir.ActivationFunctionType.Exp)

            if debug and bi == 0 and ki == 0:
                nc.sync.dma_start(dram["dP"], psl[:])
                nc.sync.dma_start(dram["dE"], esl[:])
            # ---- softmax denominator + reciprocal + A
            for h in range(2):
                dt = lps.tile([128, 512], F32, tag="lt", name="dent")
                for qi in range(4):
                    nc.tensor.matmul(dt[:, :HWO], sden_sb[:],
                                     esl_v[:, h, qi, :],
                                     start=(qi == 0), stop=(qi == 3))
                rd = mpool.tile([128, HWO], BF16, tag="rd")
                with nc.allow_low_precision(reason="softmax recip bf16"):
                    nc.vector.reciprocal(rd[:], dt[:, :HWO])
                nc.vector.tensor_mul(
                    asl_v[:, h], esl_v[:, h],
                    rd[:].unsqueeze(1).to_broadcast([128, 4, HWO]))

            if debug and bi == 0 and ki == 0:
                nc.sync.dma_start(dram["dA"], asl[:])
            # ---- replicate A over d (per o, d); spread over 4 queues
            for o in range(4):
                for dd in range(4):
                    eng = (nc.sync, nc.gpsimd, nc.scalar, nc.gpsimd)[dd]
                    eng.dma_start(arep_p[dd, :, o, :],
                                  asl[32 * o:32 * o + 32, :])

            # ---- P2 chunks + out accumulation
            for o in range(4):
                for h in range(2):
                    p2 = p2pool.tile([128, 4 * 4 * HWO], BF16, tag="p2",
                                     name="p2")
                    p2_v = p2[:].rearrange("p (q a hw) -> p q a hw", q=4, a=4)
                    nc.vector.tensor_mul(
                        p2_v[:],
                        vsl_v[:, o, h],
                        arep_v[:, o, h].unsqueeze(2)
                        .to_broadcast([128, 4, 4, HWO]))
                    if debug and bi == 0 and ki == 0 and o == 0 and h == 0:
                        nc.sync.dma_start(dram["dp2"], p2[:])
                        nc.sync.dma_start(dram["dAr"], arep[:])
                    last = (ki == nkl - 1 and o == 3 and h == 1)
                    for q in range(4):
                        for ah in range(2):
                            nc.tensor.matmul(
                                oacc[32 * q:32 * q + 32,
                                     512 * ah:512 * ah + 450],
                                so_sb[:],
                                p2_v[:, q, 2 * ah:2 * ah + 2, :],
                                start=first_acc, stop=last,
                                tile_position=(0, 32 * q))
                    first_acc = False

        # ---------------- LayerNorm over (a,d) + permute + store
        # copy oacc -> SBUF bf16 [128=(q,d,m8), (a4, hw)]
        oc = ppool.tile([128, 4 * HWO], BF16, tag="P", name="oc")
        oc_v = oc[:].rearrange("p (a hw) -> p a hw", a=4)
        nc.scalar.copy(oc[:].rearrange("p (ah a2 hw) -> p ah a2 hw",
                                       ah=2, a2=2),
                       oacc[:].rearrange("p (ah c) -> p ah c", ah=2)
                       [:, :, :450].rearrange(
                           "p ah (a2 hw) -> p ah a2 hw", a2=2))
        if debug and bi == 0:
            nc.sync.dma_start(dram["dO"], oc[:])
        sq = vpool.tile([128, 4 * HWO], BF16, tag="V", name="sq")
        nc.scalar.activation(sq[:], oc[:],
                             mybir.ActivationFunctionType.Square)
        sq_v = sq[:].rearrange("p (a hw) -> p a hw", a=4)
        mu_ps = lps.tile([128, 512], F32, tag="lt", name="mu")
        for a in range(4):
            nc.tensor.matmul(mu_ps[:, :HWO], sm_sb[:], oc_v[:, a, :],
                             start=(a == 0), stop=(a == 3))
        e2_ps = lps.tile([128, 512], F32, tag="lt", name="e2")
        for a in range(4):
            nc.tensor.matmul(e2_ps[:, :HWO], sm_sb[:], sq_v[:, a, :],
                             start=(a == 0), stop=(a == 3))
        var = mpool.tile([128, HWO], F32, tag="var", bufs=1)
        musq = mpool.tile([128, HWO], F32, tag="musq", bufs=1)
        mu = mpool.tile([128, HWO], F32, tag="mu", bufs=1)
        nc.scalar.copy(mu[:], mu_ps[:, :HWO])
        nc.vector.tensor_mul(musq[:], mu[:], mu[:])
        nc.vector.tensor_sub(var[:], e2_ps[:, :HWO], musq[:])
        sig = mpool.tile([128, HWO], F32, tag="sig", bufs=1)
        nc.scalar.activation(sig[:], var[:],
                             mybir.ActivationFunctionType.Sqrt,
                             bias=eps_sb[:, 0:1])
        rstd = mpool.tile([128, HWO], F32, tag="rstd", bufs=1)
        nc.vector.reciprocal(rstd[:], sig[:])

        t3 = mpool.tile([128, 4 * HWO], F32, tag="t3", bufs=1)
        t3_v = t3[:].rearrange("p (a hw) -> p a hw", a=4)
        nc.vector.tensor_sub(t3_v[:],
                             oc_v[:],
                             mu[:].unsqueeze(1).to_broadcast([128, 4, HWO]))
        nc.vector.tensor_mul(t3_v[:], t3_v[:],
                             rstd[:].unsqueeze(1).to_broadcast([128, 4, HWO]))
        nc.vector.tensor_mul(t3_v[:], t3_v[:],
                             gc_sb[:].unsqueeze(2).to_broadcast([128, 4, HWO]))
        nc.vector.tensor_tensor(
            t3_v[:], t3_v[:],
            bc_sb[:].unsqueeze(2).to_broadcast([128, 4, HWO]),
            op=mybir.AluOpType.add)

        # final permute: fin[m=8q+m8, (hw, 4a+d)]
        fin = spool.tile([32, HWO * 16], F32, tag="fin", name="fin")
        fin_v = fin[:].rearrange("p (hw ad) -> p hw ad", ad=16)
        sf_v = sf_sb[:].rearrange("p (d c) -> p d c", d=4)
        fin_ax = fin_v[:, :, :].rearrange("p hw (a x) -> p a hw x", a=4)
        for d in range(4):
            fp = lps.tile([128, 512], F32, tag="lt", name="fp")
            nc.tensor.matmul(fp[:32, :4 * HWO // 2], sf_v[:, d, :],
                             t3_v[:, :2, :], start=True, stop=True)
            nc.tensor.matmul(fp[32:64, :4 * HWO // 2], sf_v[:, d, :],
                             t3_v[:, 2:, :], start=True, stop=True,
                             tile_position=(0, 32))
            for e in range(2):
                nc.scalar.copy(
                    fin_ax[:, 2 * e:2 * e + 2, :, d],
                    fp[32 * e:32 * e + 32, :4 * HWO // 2].rearrange(
                        "p (a hw) -> p a hw", a=2))
        nc.sync.dma_start(dram["out"][bdi], fin[:])


# ------------------------------------------------------------------ driver
def _build_nc(nb=NB, kls=None, reps=1, debug=False):
    nc = bacc.Bacc("TRN2", target_bir_lowering=False, debug=False,
                   num_devices=NCORES)
    dram = {}
    dram["inputT"] = nc.dram_tensor("inputT", (nb, 128, 4096), BF16,
                                    kind="ExternalInput").ap()
    dram["u2"] = nc.dram_tensor("u2", (nb, 128, 4 * 4 * HWO), BF16,
                                kind="ExternalInput").ap()
    dram["wv"] = nc.dram_tensor("wv", (128, 9 * 2 * 4 * 128), BF16,
                                kind="ExternalInput").ap()
    for nm, shape, dt in (("sl", (128, 32), BF16), ("sden", (128, 128), BF16),
                          ("so32", (128, 32), BF16), ("smean", (128, 128), BF16),
                          ("sf", (4, 128, 32), F32),
                          ("gcol", (128, 4), F32), ("bcol", (128, 4), F32)):
        dram[nm] = nc.dram_tensor(nm, shape, dt, kind="ExternalInput").ap()
    dram["out"] = nc.dram_tensor("out", (nb, 32, HWO * 16), F32,
                                 kind="ExternalOutput").ap()
    if debug:
        for nm, sh, dt in (("dV", (128, 28800), BF16), ("dP", (128, 28800), BF16),
                           ("dE", (128, 1800), BF16), ("dA", (128, 1800), BF16),
                           ("dO", (128, 900), BF16), ("dp2", (128, 3600), BF16), ("dAr", (128, 7200), BF16)):
            dram[nm] = nc.dram_tensor(nm, sh, dt, kind="ExternalOutput").ap()
    with tile.TileContext(nc) as tc:
        build_program(tc, dram, nb=nb, kls=kls, reps=reps, debug=debug)
    nc.compile()
    return nc


def make_in_maps(hp):
    shared = {k: hp[k] for k in ("wv", "sl", "sden", "so32", "smean", "sf",
                                 "gcol", "bcol")}
    in_maps = []
    for c in range(NCORES):
        im = dict(shared)
        im["inputT"] = np.ascontiguousarray(hp["inputT"][c * NB:(c + 1) * NB])
        im["u2"] = np.ascontiguousarray(hp["u2"][c * NB:(c + 1) * NB])
        in_maps.append(im)
    return in_maps


def kernel(**inputs):
    input_ = np.asarray(inputs["input"], dtype=np.float32)
    ncv = np.asarray(inputs["next_capsule_value"], dtype=np.float32)
    w = np.asarray(inputs["w"], dtype=np.float32)
    gamma = np.asarray(inputs["gamma"], dtype=np.float32)
    beta = np.asarray(inputs["beta"], dtype=np.float32)

    hp = host_prep(input_, ncv, w, gamma, beta)
    nc = _build_nc()
    in_maps = make_in_maps(hp)
    from concourse.bass_utils import run_bass_kernel_spmd
    res = run_bass_kernel_spmd(nc, in_maps, core_ids=list(range(NCORES)),
                               trace=False)
    outs = res.results
    full = np.concatenate([np.asarray(o["out"]) for o in outs], axis=0)
    return full.reshape(B, M, HO, WO, DOUT).astype(np.float32)
